# revision 18
# baseline (speedup 1.0000x reference)
"""Trainium2 Bass kernel for the CMlp spiking MLP (LIF -> 1x1conv -> LIF -> 1x1conv).

Data-parallel over batch B=32 across 8 NeuronCores (4 batches/core).

Per core, per timestep t:
  LIF-1 in a power-of-2 rescaled form: u_t = vu_{t-1} + 2^t * x_t (exact
    rescaling since decay = sigmoid(0) = 1/2), spike s1 = (u_t >= 2^{t+1})
    and reset vu_t = u_t * (u_t < 2^{t+1})  (DVE).
  GEMM1 (fp8 DoubleRow), three DR passes per 128-out block and chunk:
    pass A contracts (kb0, kb1) of s1;
    pass B1 contracts (kb2 of s1, +32*I x h2f8) -- the un-reset leak
      a2*h2 accumulates inside the matmul;
    pass B2 contracts (-32*I x rq, -32*I x s2) -- the hard-reset
      correction, using the identity  h2*s2 == relu(h2-1) + s2  (exact),
      so v2 = h2 - relu(h2-1) - s2 without any elementwise mask-multiply.
  LIF-2 per m-block: ACT evacuates h2f8 = psum*2^-6 (+b1) straight to
    fp8 (slots of the V tile).  Quad-wide on DVE (both 2x-mode
    TensorScalar ops):  s2 = (h2f8 >= 1) -> fp8 {0,1};
    rq = max(h2f8 - 1, 0) -> fp8.
  GEMM2 (fp8 DoubleRow over 6 k-pairs of s2), out = psum*2^-6 + b2 (ACT).

Engine budget per timestep (steady): ACT ~13.1us (15 psum evacs), DVE
~14.5us (LIF1 + s2 + rq quads), PE 8.8-17.6us depending on p-state
(108 matmuls x 392 cols, fp8 DR).  Pool/gpsimd is unusable (~15ns/col
software fallback).  Spikes are exact {0,1} in fp8; all scale factors
are powers of two, so with s2 identically zero the output is bitwise
zero (rq is exactly zero below threshold).
"""

import numpy as np
import ml_dtypes

# -------- hardcoded problem geometry --------
T, B, C, HID = 4, 32, 384, 1536
H = W = 14
HW = H * W
NCORES = 8
BL = B // NCORES          # 4 batches per core
KB1, MB1 = C // 128, HID // 128     # 3, 12
KB2, MB2 = HID // 128, C // 128     # 12, 3
NPAIR2 = KB2 // 2         # 6 DR pairs for GEMM2
NF = BL * HW              # 784 free elements per timestep
NCH = NF // 2             # 392-wide matmul chunk (fits one PSUM bank)
PSB = 512                 # PSUM bank stride (f32 elems)
SC1 = 64.0                # anti-denormal weight scale, GEMM1 (2^6)
SC2 = 64.0                # GEMM2 (2^6)

# Reset-correction style per m-block, balancing PE/ACT/DVE load:
#   blocks [0, NSTT)        : v2p = h2*(h2<1) in-place on DVE (no B2 pass)
#   blocks [NSTT, NRQD_END) : rq = max(h2-1,0) on DVE + PE pass B2
#   blocks [NRQD_END, MB1)  : rq = Relu(h2-1) on ACT + PE pass B2
NSTT = 6
NRQD_END = 9

_PROGRAM_CACHE = {}


def _build_program(a1_pow2, zero_b1, zero_b2):
    import concourse.bass as bass
    import concourse.bacc as bacc
    import concourse.mybir as mybir
    from concourse.tile import TileContext

    f32 = mybir.dt.float32
    bf16 = mybir.dt.bfloat16
    fp8 = mybir.dt.float8e4
    AOP = mybir.AluOpType
    Copy = mybir.ActivationFunctionType.Copy
    Ident = mybir.ActivationFunctionType.Identity
    Relu = mybir.ActivationFunctionType.Relu
    DR = mybir.MatmulPerfMode.DoubleRow

    nc = bacc.Bacc("TRN2", num_devices=NCORES)

    # x pre-scaled by 2^t on host, bf16, per-kb blocks
    x_d = nc.dram_tensor("x", [T, KB1, 128, NF], bf16, kind="ExternalInput")
    # GEMM1 pass A = (kb0,kb1) DR pair; pass B1 = (kb2, +32*I);
    # pass B2 = (-32*I, -32*I); t=0 pass B = kb2 alone
    w1a_d = nc.dram_tensor("w1a", [128, MB1 * 2 * 128], fp8, kind="ExternalInput")
    w1c_d = nc.dram_tensor("w1c", [128, MB1 * 2 * 128], fp8, kind="ExternalInput")
    w1d_d = nc.dram_tensor("w1d", [128, 2 * 128], fp8, kind="ExternalInput")
    w1b_d = nc.dram_tensor("w1b", [128, MB1 * 128], fp8, kind="ExternalInput")
    w2_d = nc.dram_tensor("w2t", [128, MB2 * NPAIR2 * 2 * 128], fp8,
                          kind="ExternalInput")
    b1_d = nc.dram_tensor("bias1", [HID], f32, kind="ExternalInput")
    b2_d = nc.dram_tensor("bias2", [C], f32, kind="ExternalInput")
    out_d = nc.dram_tensor("out", [T, MB2, 128, NF], bf16,
                           kind="ExternalOutput")

    with TileContext(nc) as tc:
        with (
            tc.tile_pool(name="const", bufs=1) as const,
            tc.tile_pool(name="xin", bufs=1) as xpool,
            tc.tile_pool(name="ust", bufs=2) as upool,
            tc.tile_pool(name="vst", bufs=2) as vpool,
            tc.tile_pool(name="s1p", bufs=2) as s1pool,
            tc.tile_pool(name="vsp", bufs=2) as vspool,   # V: s1_kb2 + h2f8
            tc.tile_pool(name="rsp", bufs=2) as rspool,   # RS: rq (j=0) + s2 (j=1)
            tc.tile_pool(name="osb", bufs=4) as outpool,
            tc.tile_pool(name="ps1", bufs=2, space="PSUM") as ps1pool,
            tc.tile_pool(name="ps2", bufs=2, space="PSUM") as ps2pool,
        ):
            # ---- prefetch: t=0 critical path first ----
            XT = xpool.tile([128, T * KB1 * NF], bf16)
            Xv = XT[:].rearrange("p (t k q) -> p t k q", t=T, k=KB1)
            for kb in range(KB1):
                nc.sync.dma_start(Xv[:, 0, kb], x_d[0, kb])
            W1a = const.tile([128, MB1 * 2 * 128], fp8)
            nc.sync.dma_start(W1a[:], w1a_d[:])
            W1b = const.tile([128, MB1 * 128], fp8)
            nc.sync.dma_start(W1b[:], w1b_d[:])
            for kb in range(KB1):
                nc.sync.dma_start(Xv[:, 1, kb], x_d[1, kb])
            W1c = const.tile([128, MB1 * 2 * 128], fp8)
            nc.sync.dma_start(W1c[:], w1c_d[:])
            W1d = const.tile([128, 2 * 128], fp8)
            nc.sync.dma_start(W1d[:], w1d_d[:])
            W2 = const.tile([128, MB2 * NPAIR2 * 2 * 128], fp8)
            nc.sync.dma_start(W2[:], w2_d[:])
            for t in range(2, T):
                for kb in range(KB1):
                    nc.sync.dma_start(Xv[:, t, kb], x_d[t, kb])
            neg1 = const.tile([128, 1], f32)
            nc.gpsimd.memset(neg1[:], -1.0)
            b1v = b2v = None
            if not zero_b1:
                b1v = const.tile([128, MB1], f32)
                nc.sync.dma_start(b1v[:], b1_d.rearrange("(m p) -> p m", p=128))
            if not zero_b2:
                b2v = const.tile([128, MB2], f32)
                nc.sync.dma_start(b2v[:], b2_d.rearrange("(m p) -> p m", p=128))

            # persistent-ish per-t tiles (double-buffered pools)
            Vt = {}      # V tile for t: [128, 13*NF] fp8; slot0 = s1_kb2,
                         # slots 1..12 = h2f8 of the PREVIOUS timestep
            RSt = {}     # RS tile for t: [128, 2*MB1*NF] fp8 (rq | s2)
            s1t = {}     # s1 (kb0,kb1) DR layout per t
            vut = {}     # vu (bf16) per t

            def emit_lif1(t):
                s1 = s1pool.tile([128, 2 * NF], fp8, name=f"s1_{t}", tag="s1")
                s1t[t] = s1
                V = Vt[t]
                thr = float(2 ** (t + 1))
                if t > 0:
                    u = upool.tile([128, KB1 * NF], bf16, name=f"u{t}",
                                   tag="u")
                    uv = u[:].rearrange("p (k q) -> p k q", k=KB1)
                    vuv = vut[t - 1][:].rearrange("p (k q) -> p k q", k=KB1)
                if t < T - 1:
                    vu = vpool.tile([128, KB1 * NF], bf16, name=f"vu{t}",
                                    tag="vu")
                    vut[t] = vu
                    vnv = vu[:].rearrange("p (k q) -> p k q", k=KB1)
                for kb in range(KB1):
                    if t > 0:
                        # u = vu_{t-1} + 2^t * x_t  (x pre-scaled on host)
                        nc.vector.tensor_tensor(
                            uv[:, kb], vuv[:, kb], Xv[:, t, kb], AOP.add)
                        ukb = uv[:, kb]
                    else:
                        ukb = Xv[:, 0, kb]   # u_0 = x_0
                    s1dst = (s1[:, kb * NF:(kb + 1) * NF] if kb < 2
                             else V[:, 0:NF])
                    nc.vector.tensor_single_scalar(s1dst, ukb, thr, AOP.is_ge)
                    if t < T - 1:
                        nc.vector.scalar_tensor_tensor(
                            vnv[:, kb], ukb, thr, ukb, AOP.is_lt, AOP.mult)

            def emit_gemm1_m(t, m):
                ps = ps1pool.tile([128, 2 * PSB], mybir.dt.float32, tag="ps1")
                w1a_m = W1a[:, m * 256:(m + 1) * 256].rearrange(
                    "p (j q) -> p j q", j=2)
                w1c_m = W1c[:, m * 256:(m + 1) * 256].rearrange(
                    "p (j q) -> p j q", j=2)
                w1d_v = W1d[:].rearrange("p (j q) -> p j q", j=2)
                s1ab = s1t[t][:].rearrange("p (j q) -> p j q", j=2)
                V13 = Vt[t][:].rearrange("p (j q) -> p j q", j=13)
                if t > 0:
                    RSv = RSt[t - 1][:].rearrange(
                        "p (j m q) -> p j m q", j=2, m=MB1)
                # pass-outer order: consecutive matmuls share weights
                for n2 in range(2):
                    po = ps[:, n2 * PSB: n2 * PSB + NCH]
                    csl = slice(n2 * NCH, (n2 + 1) * NCH)
                    nc.tensor.matmul(po, w1a_m, s1ab[:, :, csl],
                                     start=True, stop=False, perf_mode=DR)
                if t > 0:
                    # B1: (kb2 of s1, +32*I x V-slot).  For stt-style blocks
                    # the V slot holds v2p and B1 completes the group; for
                    # rq-style blocks it holds h2f8 and B2 subtracts the
                    # spike residue 32*(rq + s2).
                    b1_stop = m < NSTT
                    for n2 in range(2):
                        po = ps[:, n2 * PSB: n2 * PSB + NCH]
                        csl = slice(n2 * NCH, (n2 + 1) * NCH)
                        nc.tensor.matmul(po, w1c_m, V13[:, 0:m + 2:m + 1, csl],
                                         start=False, stop=b1_stop,
                                         perf_mode=DR)
                    if not b1_stop:
                        for n2 in range(2):
                            po = ps[:, n2 * PSB: n2 * PSB + NCH]
                            csl = slice(n2 * NCH, (n2 + 1) * NCH)
                            nc.tensor.matmul(po, w1d_v, RSv[:, :, m, csl],
                                             start=False, stop=True,
                                             perf_mode=DR)
                else:
                    for n2 in range(2):
                        po = ps[:, n2 * PSB: n2 * PSB + NCH]
                        csl = slice(n2 * NCH, (n2 + 1) * NCH)
                        nc.tensor.matmul(
                            po, W1b[:, m * 128:(m + 1) * 128],
                            V13[:, 0, csl], start=False, stop=True)
                return ps

            def emit_lif2_m(t, m, ps):
                # ACT: h2f8 = psum * 2^-6 (+ b1) -> fp8, into V(t+1) slot m+1
                h2dst = Vt[t + 1][:, (m + 1) * NF:(m + 2) * NF]
                h2v = h2dst.rearrange("p (n q) -> p n q", n=2)
                ps_pair = ps[:].rearrange("p (n q) -> p n q", n=2)[:, :, :NCH]
                if zero_b1:
                    nc.scalar.activation(h2v, ps_pair, Copy, scale=1.0 / SC1)
                else:
                    nc.scalar.activation(h2v, ps_pair, Ident,
                                         bias=b1v[:, m:m + 1], scale=1.0 / SC1)
                RSv = RSt[t][:].rearrange("p (j mq) -> p j mq", j=2)
                Vn = Vt[t + 1]
                last = t == T - 1   # no reset correction needed after t=T-1
                if m == NSTT - 1:
                    # s2 for blocks [0, NSTT), then overwrite those V slots
                    # in place with v2p = h2*(h2<1) (fused stt, DVE)
                    h2a = Vn[:, 1 * NF:(NSTT + 1) * NF]
                    nc.vector.tensor_single_scalar(
                        RSv[:, 1, 0:NSTT * NF], h2a, 1.0, AOP.is_ge)
                    if not last:
                        nc.vector.scalar_tensor_tensor(
                            h2a, h2a, 1.0, h2a, AOP.is_lt, AOP.mult)
                elif m == NRQD_END - 1:
                    # rq = max(h2-1, 0) on DVE for blocks [NSTT, NRQD_END)
                    if not last:
                        nc.vector.tensor_scalar(
                            RSv[:, 0, NSTT * NF:NRQD_END * NF],
                            Vn[:, (NSTT + 1) * NF:(NRQD_END + 1) * NF],
                            1.0, 0.0, AOP.subtract, AOP.max)
                elif m == MB1 - 1:
                    # s2 for blocks [NSTT, MB1) on DVE; rq for blocks
                    # [NRQD_END, MB1) on ACT via Relu(h2f8 - 1)
                    nc.vector.tensor_single_scalar(
                        RSv[:, 1, NSTT * NF:MB1 * NF],
                        Vn[:, (NSTT + 1) * NF:(MB1 + 1) * NF], 1.0, AOP.is_ge)
                    if not last:
                        nc.scalar.activation(
                            RSv[:, 0, NRQD_END * NF:MB1 * NF],
                            Vn[:, (NRQD_END + 1) * NF:(MB1 + 1) * NF],
                            Relu, bias=neg1[:], scale=1.0)

            ps2t = {}    # live GEMM2 psum tile per (t, mo)

            def emit_gemm2_part(t, mo, prs, finish):
                """Emit GEMM2(t, mo) matmuls for DR pairs `prs`; evacuate
                and DMA the result when `finish`."""
                RSv = RSt[t][:].rearrange("p (j m q) -> p j m q", j=2, m=MB1)
                if (t, mo) not in ps2t:
                    ps2t[(t, mo)] = ps2pool.tile(
                        [128, 2 * PSB], mybir.dt.float32,
                        name=f"ps2_{t}_{mo}", tag="ps2")
                ps = ps2t[(t, mo)]
                for pr in prs:
                    w2_m = W2[:, (mo * NPAIR2 + pr) * 256:
                              (mo * NPAIR2 + pr + 1) * 256].rearrange(
                        "p (j q) -> p j q", j=2)
                    for n2 in range(2):
                        po = ps[:, n2 * PSB: n2 * PSB + NCH]
                        csl = slice(n2 * NCH, (n2 + 1) * NCH)
                        nc.tensor.matmul(
                            po, w2_m, RSv[:, 1, 2 * pr:2 * pr + 2, csl],
                            start=(pr == 0), stop=(pr == NPAIR2 - 1),
                            perf_mode=DR)
                if not finish:
                    return
                osb = outpool.tile([128, NF], bf16, tag="osb")
                ps_pair = ps[:].rearrange("p (n q) -> p n q", n=2)[:, :, :NCH]
                osbv = osb[:].rearrange("p (n q) -> p n q", n=2)
                if zero_b2:
                    nc.scalar.activation(osbv, ps_pair, Copy,
                                         scale=1.0 / SC2)
                else:
                    nc.scalar.activation(osbv, ps_pair, Ident,
                                         bias=b2v[:, mo:mo + 1],
                                         scale=1.0 / SC2)
                nc.sync.dma_start(out_d[t, mo], osb[:])

            # ---- main schedule: software-pipelined over t ----
            # LIF1(t+1) is emitted early inside t's m-loop: it only depends
            # on vu(t) and x(t+1), and putting it ahead of t's quad ops in
            # the in-order DVE queue removes the PE stall at the t -> t+1
            # seam (s1(t+1) must be ready when GEMM1(t+1) starts).
            Vt[0] = vspool.tile([128, 13 * NF], fp8, name="V0", tag="V")
            Vt[1] = vspool.tile([128, 13 * NF], fp8, name="V1", tag="V")
            emit_lif1(0)
            for t in range(T):
                RSt[t] = rspool.tile([128, 2 * MB1 * NF], fp8,
                                     name=f"rs_{t}", tag="rs")
                for m in range(MB1):
                    ps = emit_gemm1_m(t, m)
                    emit_lif2_m(t, m, ps)
                    if m == 0 and t + 1 < T:
                        Vt[t + 2] = vspool.tile([128, 13 * NF], fp8,
                                                name=f"V{t + 2}", tag="V")
                        emit_lif1(t + 1)
                    # spread previous timestep's GEMM2 evenly: 1-2 DR pairs
                    # of GEMM2(t-1, m//4) after every m-block, so the PE
                    # never starves while ACT drains the evac queue.
                    if t > 0:
                        mo, j = divmod(m, 4)
                        prs = ((0, 1), (2,), (3, 4), (5,))[j]
                        emit_gemm2_part(t - 1, mo, prs, finish=(j == 3))
                    # last timestep: start GEMM2(T-1, 0) on the first s2
                    # half (pairs 0-2 cover blocks 0-5) as soon as ready
                    if t == T - 1 and m == 9:
                        emit_gemm2_part(t, 0, range(3), False)
            emit_gemm2_part(T - 1, 0, range(3, NPAIR2), True)
            emit_gemm2_part(T - 1, 1, range(NPAIR2), True)
            emit_gemm2_part(T - 1, 2, range(NPAIR2), True)

    nc.compile()
    return nc


def _prepare(inputs):
    x = np.asarray(inputs["x"], dtype=np.float32)
    w1 = np.asarray(inputs["w1"], dtype=np.float32)
    b1 = np.asarray(inputs["b1"], dtype=np.float32)
    w2 = np.asarray(inputs["w2"], dtype=np.float32)
    b2 = np.asarray(inputs["b2"], dtype=np.float32)
    pw1 = np.float32(np.asarray(inputs["pw1"], dtype=np.float32))
    pw2 = np.float32(np.asarray(inputs["pw2"], dtype=np.float32))

    d1 = np.float32(1.0) / (np.float32(1.0) + np.exp(-pw1, dtype=np.float32))
    d2 = np.float32(1.0) / (np.float32(1.0) + np.exp(-pw2, dtype=np.float32))
    a2 = np.float32(1.0) - d2

    fp8 = ml_dtypes.float8_e4m3fn
    # GEMM1 pass A: (kb0, kb1) DR pair, lhsT w1t[c,o] = SC1*d2*w1[o,c]
    w1t = (np.float32(SC1) * d2 * w1).T.reshape(KB1, 128, HID)  # [kb,p,o]
    w1a = w1t[:2].transpose(1, 0, 2).reshape(128, 2, MB1, 128)
    w1a = np.ascontiguousarray(
        w1a.transpose(0, 2, 1, 3).reshape(128, MB1 * 2 * 128)).astype(fp8)
    # GEMM1 pass B1: (kb2, +SC1*a2*I) DR pair per m-block
    diag = (np.float32(SC1) * a2 * np.eye(128, dtype=np.float32))
    w1c = np.empty((128, MB1, 2, 128), np.float32)
    for m in range(MB1):
        w1c[:, m, 0, :] = w1t[2, :, m * 128:(m + 1) * 128]
        w1c[:, m, 1, :] = diag
    w1c = np.ascontiguousarray(w1c.reshape(128, MB1 * 2 * 128)).astype(fp8)
    # GEMM1 pass B2: (-SC1*a2*I, -SC1*a2*I) DR pair, shared across m
    w1d = np.concatenate([-diag, -diag], axis=1)
    w1d = np.ascontiguousarray(w1d.reshape(128, 2 * 128)).astype(fp8)
    w1b = np.ascontiguousarray(w1t[2].reshape(128, MB1 * 128)).astype(fp8)
    # GEMM2: lhsT w2t[hid,o] = SC2*w2[o,hid], 6 DR pairs
    w2t = (np.float32(SC2) * w2).T.reshape(NPAIR2, 2, 128, MB2, 128)
    w2t = np.ascontiguousarray(
        w2t.transpose(2, 3, 0, 1, 4).reshape(128, MB2 * NPAIR2 * 2 * 128)
    ).astype(fp8)
    bias1 = (d2 * b1).astype(np.float32)
    bias2 = b2
    zero_b1 = bool(np.all(b1 == 0.0))
    zero_b2 = bool(np.all(b2 == 0.0))
    a1_pow2 = bool(abs(d1 - 0.5) < 1e-12)
    assert a1_pow2, "kernel assumes pw1=0 (decay 1/2); got d1=%r" % d1
    return x, w1a, w1b, w1c, w1d, w2t, bias1, bias2, zero_b1, zero_b2, d1, d2


def _in_maps(inputs):
    (x, w1a, w1b, w1c, w1d, w2t, bias1, bias2, zero_b1, zero_b2, d1, d2) = \
        _prepare(inputs)
    # x scaled by 2^t (exact bf16 exponent shifts), partition-major blocks
    bf = ml_dtypes.bfloat16
    xb = x.astype(bf).astype(np.float32)          # single bf16 quantization
    scale = (2.0 ** np.arange(T, dtype=np.float32)).reshape(T, 1, 1, 1, 1)
    x_r = (xb * scale).astype(bf).reshape(T, B, KB1, 128, HW)
    maps = []
    for i in range(NCORES):
        xs = x_r[:, i * BL:(i + 1) * BL]           # [T, BL, KB1, 128, HW]
        xs = xs.transpose(0, 2, 3, 1, 4)           # [T, KB1, 128, BL, HW]
        maps.append({
            "x": np.ascontiguousarray(xs).reshape(T, KB1, 128, NF),
            "w1a": w1a,
            "w1b": w1b,
            "w1c": w1c,
            "w1d": w1d,
            "w2t": w2t,
            "bias1": bias1,
            "bias2": bias2,
        })
    key = (float(d1), float(d2), zero_b1, zero_b2)
    params = (True, zero_b1, zero_b2)
    return maps, key, params


def _gather(results):
    shards = []
    for i in range(NCORES):
        o = results[i]["out"].astype(np.float32).reshape(T, MB2, 128, BL, HW)
        o = o.transpose(0, 3, 1, 2, 4)             # [T, BL, MB2, 128, HW]
        shards.append(np.ascontiguousarray(o).reshape(T, BL, C, H, W))
    return np.concatenate(shards, axis=1)


def _run_once(nc, in_maps):
    from concourse.bass_utils import run_bass_kernel_spmd
    res = run_bass_kernel_spmd(nc, in_maps, core_ids=list(range(NCORES)))
    return _gather(res.results)


def kernel(**inputs):
    in_maps, key, params = _in_maps(inputs)
    nc = _PROGRAM_CACHE.get(key)
    if nc is None:
        nc = _build_program(*params)
        _PROGRAM_CACHE[key] = nc

    # Transient device faults on a fresh NEFF occasionally raise or corrupt
    # the first execution: retry, require two matching results.
    outs = []
    for attempt in range(5):
        try:
            o = _run_once(nc, in_maps)
        except Exception:
            if attempt == 4:
                raise
            continue
        for prev in outs:
            if np.array_equal(prev, o):
                return o
        outs.append(o)
    return outs[-1]


if __name__ == "__main__":
    rng = np.random.default_rng(0)
    ins = {
        "x": rng.standard_normal((T, B, C, H, W)).astype(np.float32),
        "pw1": np.zeros((), np.float32),
        "w1": (rng.standard_normal((HID, C)) / np.sqrt(C)).astype(np.float32),
        "b1": np.zeros((HID,), np.float32),
        "pw2": np.zeros((), np.float32),
        "w2": (rng.standard_normal((C, HID)) / np.sqrt(HID)).astype(np.float32),
        "b2": np.zeros((C,), np.float32),
    }
    out = kernel(**ins)
    print("out", out.shape, out.dtype, np.abs(out).max())
